# revision 21
# baseline (speedup 1.0000x reference)
"""Bass/Trainium2 kernel for nn_BinResNetConv2d.

Computes: BatchNorm2d (inference) -> sign binarization -> 3x3 conv
(256->256 ch, stride 1, pad 1, no bias) -> ReLU on x[32, 256, 56, 56].

Strategy: data-parallel over batch across 8 NeuronCores (4 images/core,
conv weights + BN params replicated). Per core:
  - BN is folded on host into per-channel (scale, shift); on device one
    ScalarE activation computes sign(x*scale + shift) per tile, writing
    bf16 +/-1 into a zero-padded [128ci, 58, 58] SBUF image.
  - The 3x3 conv is 18 accumulating bf16 matmuls per output tile
    (9 taps x 2 ci-tiles of 128), K=128 on partitions, moving operand
    N = 8 rows x 56 cols = 448, accumulated in one PSUM bank.
  - ReLU on VectorE evacuates PSUM -> SBUF f32, DMA to HBM.

Weights are pre-transposed on host to lhsT layout [ci, tap, co] and
rounded to bf16 (sign inputs are exactly +/-1 in bf16, so the only
error is the bf16 weight rounding, ~1.5e-3 scale-relative absmax).
Set N_SPLIT = 2 for a hi/lo bf16 weight split (~2.5e-6 error, 2x PE
cost).
"""

import numpy as np
import ml_dtypes

N_CORES = 8
NB = 4            # images per core (32 / 8)
C = 256
H = W = 56
HP = WP = 58      # padded spatial
N_SPLIT = 1       # 1 = single bf16 weight pass, 2 = hi/lo split
ROWS_PER_TILE = 8
N_ROW_TILES = H // ROWS_PER_TILE  # 7

_nc_cache = {}
LAST_RESULTS = None


def _build_nc():
    import concourse.mybir as mybir
    import concourse.tile as tile
    from concourse import bacc

    f32 = mybir.dt.float32
    bf16 = mybir.dt.bfloat16
    AF = mybir.ActivationFunctionType

    nc = bacc.Bacc("TRN2", target_bir_lowering=False, debug=False)
    x_d = nc.dram_tensor("x", (NB, C, H, W), f32, kind="ExternalInput")
    # wt[s, ci_t, co_t, ci, tap, co]: lhsT slices, co-half-major so the
    # first-needed co half can be DMA'd on its own
    wt_d = nc.dram_tensor("wt", (N_SPLIT, 2, 2, 128, 9, 128), bf16,
                          kind="ExternalInput")
    bnp_d = nc.dram_tensor("bnp", (2, 128, 2), f32, kind="ExternalInput")
    y_d = nc.dram_tensor("y", (NB, C, H, W), f32, kind="ExternalOutput")

    with tile.TileContext(nc) as tc:
        with (
            tc.tile_pool(name="const", bufs=1) as cpool,
            tc.tile_pool(name="xp", bufs=1) as xpool,
            tc.tile_pool(name="chunk", bufs=8) as hpool,
            tc.tile_pool(name="stage", bufs=3) as spool,
            tc.tile_pool(name="out", bufs=4) as opool,
            tc.tile_pool(name="psum", bufs=8, space="PSUM") as ppool,
        ):
            # zero scratch for PE warm-up matmuls (HAM un-throttles after
            # ~3.4us of sustained PE work; run it on zeros while x loads).
            # First gpsimd op so the warm-up can start right after boot.
            warm_sb = cpool.tile([128, 64], f32, tag="warm")
            nc.gpsimd.memset(warm_sb[:], 0.0)
            # BN params first on the SP HW-DGE ring (tiny, lands ~instantly;
            # the ScalarE FIFO waits on this before its first Sign)
            bnp_sb = []  # [128, 2]: col 0 = scale, col 1 = shift
            for ci_t in range(2):
                t = cpool.tile([128, 2], f32, tag=f"bnp{ci_t}")
                nc.sync.dma_start(t[:], bnp_d[ci_t])
                bnp_sb.append(t)
            # weights on the ScalarE HW-DGE ring, first-needed co half first;
            # lands in parallel with the x chunk loads on the SP ring
            wt_sb = {}   # (split, ci_t) -> [128, 2, 9, 128] bf16 (co_t major)
            for s in range(N_SPLIT):
                for ci_t in range(2):
                    wt_sb[(s, ci_t)] = cpool.tile(
                        [128, 2, 9, 128], bf16, tag=f"wt{s}_{ci_t}",
                        name=f"wt{s}_{ci_t}")
            for co_t in range(2):
                for s in range(N_SPLIT):
                    for ci_t in range(2):
                        nc.scalar.dma_start(
                            wt_sb[(s, ci_t)][:, co_t], wt_d[s, ci_t, co_t])

            # --- padded bf16 images; borders zeroed (disjoint from the
            # interior Sign writes, so no dep lands on the Activation ops)
            xp = {}   # (n, ci_t) -> [128, 58, 58] bf16, zero border
            for n in range(NB):
                for ci_t in range(2):
                    t = xpool.tile([128, HP, WP], bf16, tag=f"xp{n}_{ci_t}")
                    nc.gpsimd.memset(t[:, 0, :], 0.0)
                    nc.gpsimd.memset(t[:, HP - 1, :], 0.0)
                    nc.gpsimd.memset(t[:, 1:HP - 1, 0], 0.0)
                    nc.gpsimd.memset(t[:, 1:HP - 1, WP - 1], 0.0)
                    xp[(n, ci_t)] = t

            def binarize(ci_t, dst_rows, src_ap):
                """Sign(x*scale+shift) into padded rows [dst_rows] of xp."""
                nc.scalar.activation(
                    dst_rows, src_ap, AF.Sign,
                    bias=bnp_sb[ci_t][:, 1:2], scale=bnp_sb[ci_t][:, 0:1])

            # image 0 in row-chunks per ci tile: first conv matmuls can
            # start as soon as the first ~7 rows have landed + signed.
            # Chunk order (c, ci) so the earliest-needed rows arrive first.
            CHUNK_ROWS = [8, 6, 14, 14, 14]
            chunk0 = None
            r = 0
            for nr in CHUNK_ROWS:
                for ci_t in range(2):
                    st = hpool.tile([128, 14, W], f32, tag="chunk")
                    nc.sync.dma_start(
                        st[:, 0:nr, :],
                        x_d[0, ci_t * 128:(ci_t + 1) * 128, r:r + nr, :])
                    binarize(ci_t,
                             xp[(0, ci_t)][:, 1 + r:1 + r + nr, 1:WP - 1],
                             st[:, 0:nr, :])
                    if chunk0 is None:
                        chunk0 = st
                r += nr

            # PE warm-up: zero matmuls keep the PE's activity monitor busy
            # from ~7us until the first real matmul, so conv starts at the
            # full 2.4GHz clock instead of the 1.2GHz cold state
            warm_ps = ppool.tile([64, 64], f32, tag="ps")
            last_warm = None
            for _ in range(13):
                last_warm = nc.tensor.matmul(
                    warm_ps[:], warm_sb[:, 0:64], warm_sb[:, 0:64])
            for _ in range(6):    # bridge: runs once the first x chunk lands
                last_warm = nc.tensor.matmul(
                    warm_ps[:, 0:56], warm_sb[:, 0:64], chunk0[:, 0, 0:56])

            # images 1..3: whole-tile loads
            for n in range(1, NB):
                for ci_t in range(2):
                    st = spool.tile([128, H, W], f32, tag="stage")
                    nc.sync.dma_start(
                        st[:], x_d[n, ci_t * 128:(ci_t + 1) * 128])
                    binarize(ci_t,
                             xp[(n, ci_t)][:, 1:HP - 1, 1:WP - 1], st[:])

            # --- conv: 18*N_SPLIT accumulating matmuls per output tile ---
            from concourse.tile import add_dep_helper

            n_acc = 18 * N_SPLIT
            n_tiles = NB * 2 * N_ROW_TILES
            ti = 0
            first_mm = None
            for n in range(NB):
                for co_t in range(2):
                    co_sl = slice(co_t * 128, (co_t + 1) * 128)
                    for rb in range(N_ROW_TILES):
                        r0 = rb * ROWS_PER_TILE
                        ps = ppool.tile([128, ROWS_PER_TILE, W], f32, tag="ps")
                        k = 0
                        # ky outer: the first matmuls of image 0 only need
                        # the first x row-chunk to have landed
                        for s in range(N_SPLIT):
                            for ky in range(3):
                                for ci_t in range(2):
                                    for kx in range(3):
                                        lhsT = wt_sb[(s, ci_t)][
                                            :, co_t, ky * 3 + kx, :]
                                        rhs = xp[(n, ci_t)][
                                            :, r0 + ky:r0 + ky + ROWS_PER_TILE,
                                            kx:kx + W]
                                        mm = nc.tensor.matmul(
                                            ps[:], lhsT, rhs,
                                            start=(k == 0),
                                            stop=(k == n_acc - 1))
                                        if first_mm is None:
                                            first_mm = mm
                                        k += 1
                        ob = opool.tile([128, ROWS_PER_TILE, W], f32, tag="ob")
                        nc.vector.tensor_scalar_max(ob[:], ps[:], 0.0)
                        ti += 1
                        if ti >= n_tiles - 1:
                            # split the final stores across two queues so the
                            # kernel tail isn't one serial 229KB transfer
                            half = ROWS_PER_TILE // 2
                            nc.sync.dma_start(
                                y_d[n, co_sl, r0:r0 + half, :],
                                ob[:, 0:half, :])
                            nc.sync.dma_start(
                                y_d[n, co_sl, r0 + half:r0 + ROWS_PER_TILE, :],
                                ob[:, half:ROWS_PER_TILE, :])
                        else:
                            nc.sync.dma_start(
                                y_d[n, co_sl, r0:r0 + ROWS_PER_TILE, :], ob[:])
            # keep warm-up strictly before the real matmuls on the PE queue
            add_dep_helper(first_mm.ins, last_warm.ins, sync=False,
                           reason="PE warm-up precedes conv")
    nc.compile()
    return nc


def _get_nc():
    if "nc" not in _nc_cache:
        _nc_cache["nc"] = _build_nc()
    return _nc_cache["nc"]


def kernel(x, w, gamma, beta, running_mean, running_var, _trace=False):
    global LAST_RESULTS
    from concourse.bass_utils import run_bass_kernel_spmd

    x = np.ascontiguousarray(np.asarray(x, dtype=np.float32))
    w = np.asarray(w, dtype=np.float32)
    gamma = np.asarray(gamma, dtype=np.float32)
    beta = np.asarray(beta, dtype=np.float32)
    running_mean = np.asarray(running_mean, dtype=np.float32)
    running_var = np.asarray(running_var, dtype=np.float32)

    # fold BN (inference) into per-channel scale/shift
    eps = 1e-5
    scale = gamma / np.sqrt(running_var + eps)
    shift = beta - running_mean * scale

    # weights -> lhsT layout [ci_t, co_t, ci, (ky,kx), co], bf16
    # (optionally hi/lo split)
    wt_f32 = (w.transpose(1, 2, 3, 0)            # [ci, ky, kx, co]
              .reshape(2, 128, 9, 2, 128)        # [ci_t, ci, tap, co_t, co]
              .transpose(0, 3, 1, 2, 4))         # [ci_t, co_t, ci, tap, co]
    parts = []
    rem = wt_f32
    for _ in range(N_SPLIT):
        p = rem.astype(ml_dtypes.bfloat16)
        parts.append(p)
        rem = rem - p.astype(np.float32)
    wt = np.ascontiguousarray(np.stack(parts, axis=0))

    nc = _get_nc()
    bnp = np.ascontiguousarray(
        np.stack([scale, shift], axis=-1).reshape(2, 128, 2).astype(np.float32))
    in_maps = [
        {
            "x": np.ascontiguousarray(x[i * NB:(i + 1) * NB]),
            "wt": wt,
            "bnp": bnp,
        }
        for i in range(N_CORES)
    ]
    res = run_bass_kernel_spmd(nc, in_maps, core_ids=list(range(N_CORES)),
                               trace=_trace)
    LAST_RESULTS = res
    y = np.concatenate([r["y"] for r in res.results], axis=0)
    return y


# revision 23
# speedup vs baseline: 1.0028x; 1.0028x over previous
"""Bass/Trainium2 kernel for nn_BinResNetConv2d.

Computes: BatchNorm2d (inference) -> sign binarization -> 3x3 conv
(256->256 ch, stride 1, pad 1, no bias) -> ReLU on x[32, 256, 56, 56].

Strategy: data-parallel over batch across 8 NeuronCores (4 images/core,
conv weights + BN params replicated). Per core:
  - BN is folded on host into per-channel (scale, shift); on device one
    ScalarE activation computes sign(x*scale + shift) per tile, writing
    bf16 +/-1 into a zero-padded [128ci, 58, 58] SBUF image.
  - The 3x3 conv is 18 accumulating bf16 matmuls per output tile
    (9 taps x 2 ci-tiles of 128), K=128 on partitions, moving operand
    N = 8 rows x 56 cols = 448, accumulated in one PSUM bank.
  - ReLU on VectorE evacuates PSUM -> SBUF f32, DMA to HBM.

Weights are pre-transposed on host to lhsT layout [ci, tap, co] and
rounded to bf16 (sign inputs are exactly +/-1 in bf16, so the only
error is the bf16 weight rounding, ~1.5e-3 scale-relative absmax).
Set N_SPLIT = 2 for a hi/lo bf16 weight split (~2.5e-6 error, 2x PE
cost).
"""

import numpy as np
import ml_dtypes

N_CORES = 8
NB = 4            # images per core (32 / 8)
C = 256
H = W = 56
HP = WP = 58      # padded spatial
N_SPLIT = 1       # 1 = single bf16 weight pass, 2 = hi/lo split
ROWS_PER_TILE = 8
N_ROW_TILES = H // ROWS_PER_TILE  # 7

_nc_cache = {}
LAST_RESULTS = None


def _build_nc():
    import concourse.mybir as mybir
    import concourse.tile as tile
    from concourse import bacc

    f32 = mybir.dt.float32
    bf16 = mybir.dt.bfloat16
    AF = mybir.ActivationFunctionType

    nc = bacc.Bacc("TRN2", target_bir_lowering=False, debug=False)
    x_d = nc.dram_tensor("x", (NB, C, H, W), f32, kind="ExternalInput")
    # wt[s, ci_t, co_t, ci, tap, co]: lhsT slices, co-half-major so the
    # first-needed co half can be DMA'd on its own
    wt_d = nc.dram_tensor("wt", (N_SPLIT, 2, 2, 128, 9, 128), bf16,
                          kind="ExternalInput")
    bnp_d = nc.dram_tensor("bnp", (2, 128, 2), f32, kind="ExternalInput")
    y_d = nc.dram_tensor("y", (NB, C, H, W), f32, kind="ExternalOutput")

    with tile.TileContext(nc) as tc:
        with (
            tc.tile_pool(name="const", bufs=1) as cpool,
            tc.tile_pool(name="xp", bufs=1) as xpool,
            tc.tile_pool(name="chunk", bufs=8) as hpool,
            tc.tile_pool(name="stage", bufs=3) as spool,
            tc.tile_pool(name="out", bufs=4) as opool,
            tc.tile_pool(name="psum", bufs=8, space="PSUM") as ppool,
        ):
            # zero scratch for PE warm-up matmuls (HAM un-throttles after
            # ~3.4us of sustained PE work; run it on zeros while x loads).
            # First gpsimd op so the warm-up can start right after boot.
            warm_sb = cpool.tile([128, 256], bf16, tag="warm")
            nc.gpsimd.memset(warm_sb[:], 0.0)
            # BN params first on the SP HW-DGE ring (tiny, lands ~instantly;
            # the ScalarE FIFO waits on this before its first Sign)
            bnp_sb = []  # [128, 2]: col 0 = scale, col 1 = shift
            for ci_t in range(2):
                t = cpool.tile([128, 2], f32, tag=f"bnp{ci_t}")
                nc.sync.dma_start(t[:], bnp_d[ci_t])
                bnp_sb.append(t)
            # weights on the ScalarE HW-DGE ring, first-needed co half first;
            # lands in parallel with the x chunk loads on the SP ring
            wt_sb = {}   # (split, ci_t) -> [128, 2, 9, 128] bf16 (co_t major)
            for s in range(N_SPLIT):
                for ci_t in range(2):
                    wt_sb[(s, ci_t)] = cpool.tile(
                        [128, 2, 9, 128], bf16, tag=f"wt{s}_{ci_t}",
                        name=f"wt{s}_{ci_t}")
            for co_t in range(2):
                for s in range(N_SPLIT):
                    for ci_t in range(2):
                        nc.scalar.dma_start(
                            wt_sb[(s, ci_t)][:, co_t], wt_d[s, ci_t, co_t])

            # --- padded bf16 images; borders zeroed (disjoint from the
            # interior Sign writes, so no dep lands on the Activation ops)
            xp = {}   # (n, ci_t) -> [128, 58, 58] bf16, zero border
            for n in range(NB):
                for ci_t in range(2):
                    t = xpool.tile([128, HP, WP], bf16, tag=f"xp{n}_{ci_t}")
                    nc.gpsimd.memset(t[:, 0, :], 0.0)
                    nc.gpsimd.memset(t[:, HP - 1, :], 0.0)
                    nc.gpsimd.memset(t[:, 1:HP - 1, 0], 0.0)
                    nc.gpsimd.memset(t[:, 1:HP - 1, WP - 1], 0.0)
                    xp[(n, ci_t)] = t

            def binarize(ci_t, dst_rows, src_ap):
                """Sign(x*scale+shift) into padded rows [dst_rows] of xp."""
                nc.scalar.activation(
                    dst_rows, src_ap, AF.Sign,
                    bias=bnp_sb[ci_t][:, 1:2], scale=bnp_sb[ci_t][:, 0:1])

            # image 0 in row-chunks per ci tile: first conv matmuls can
            # start as soon as the first ~7 rows have landed + signed.
            # Chunk order (c, ci) so the earliest-needed rows arrive first.
            CHUNK_ROWS = [8, 6, 14, 14, 14]
            chunk0 = None
            r = 0
            for nr in CHUNK_ROWS:
                for ci_t in range(2):
                    st = hpool.tile([128, 14, W], f32, tag="chunk")
                    nc.sync.dma_start(
                        st[:, 0:nr, :],
                        x_d[0, ci_t * 128:(ci_t + 1) * 128, r:r + nr, :])
                    binarize(ci_t,
                             xp[(0, ci_t)][:, 1 + r:1 + r + nr, 1:WP - 1],
                             st[:, 0:nr, :])
                    if chunk0 is None:
                        chunk0 = st
                r += nr

            # PE warm-up: zero matmuls keep the PE's activity monitor busy
            # from ~7us until the first real matmul, so conv starts at the
            # full 2.4GHz clock instead of the 1.2GHz cold state
            warm_ps = ppool.tile([128, 448], f32, tag="ps")
            last_warm = None
            for _ in range(16):
                last_warm = nc.tensor.matmul(
                    warm_ps[0:64, 0:256], warm_sb[:, 0:64], warm_sb[:])
            for _ in range(4):    # bridge: runs once the weights land
                last_warm = nc.tensor.matmul(
                    warm_ps[:, 0:384], wt_sb[(0, 0)][:, 0, 0, :],
                    wt_sb[(0, 0)][:, 0, 0:3, :])

            # images 1..3: whole-tile loads
            for n in range(1, NB):
                for ci_t in range(2):
                    st = spool.tile([128, H, W], f32, tag="stage")
                    nc.sync.dma_start(
                        st[:], x_d[n, ci_t * 128:(ci_t + 1) * 128])
                    binarize(ci_t,
                             xp[(n, ci_t)][:, 1:HP - 1, 1:WP - 1], st[:])

            # --- conv: 18*N_SPLIT accumulating matmuls per output tile ---
            from concourse.tile import add_dep_helper

            n_acc = 18 * N_SPLIT
            n_tiles = NB * 2 * N_ROW_TILES
            ti = 0
            first_mm = None
            for n in range(NB):
                for co_t in range(2):
                    co_sl = slice(co_t * 128, (co_t + 1) * 128)
                    for rb in range(N_ROW_TILES):
                        r0 = rb * ROWS_PER_TILE
                        ps = ppool.tile([128, ROWS_PER_TILE, W], f32, tag="ps")
                        k = 0
                        # ky outer: the first matmuls of image 0 only need
                        # the first x row-chunk to have landed
                        for s in range(N_SPLIT):
                            for ky in range(3):
                                for ci_t in range(2):
                                    for kx in range(3):
                                        lhsT = wt_sb[(s, ci_t)][
                                            :, co_t, ky * 3 + kx, :]
                                        rhs = xp[(n, ci_t)][
                                            :, r0 + ky:r0 + ky + ROWS_PER_TILE,
                                            kx:kx + W]
                                        mm = nc.tensor.matmul(
                                            ps[:], lhsT, rhs,
                                            start=(k == 0),
                                            stop=(k == n_acc - 1))
                                        if first_mm is None:
                                            first_mm = mm
                                        k += 1
                        ob = opool.tile([128, ROWS_PER_TILE, W], f32, tag="ob")
                        nc.vector.tensor_scalar_max(ob[:], ps[:], 0.0)
                        ti += 1
                        if ti >= n_tiles - 1:
                            # split the final stores across two queues so the
                            # kernel tail isn't one serial 229KB transfer
                            half = ROWS_PER_TILE // 2
                            nc.sync.dma_start(
                                y_d[n, co_sl, r0:r0 + half, :],
                                ob[:, 0:half, :])
                            nc.sync.dma_start(
                                y_d[n, co_sl, r0 + half:r0 + ROWS_PER_TILE, :],
                                ob[:, half:ROWS_PER_TILE, :])
                        else:
                            nc.sync.dma_start(
                                y_d[n, co_sl, r0:r0 + ROWS_PER_TILE, :], ob[:])
            # keep warm-up strictly before the real matmuls on the PE queue
            add_dep_helper(first_mm.ins, last_warm.ins, sync=False,
                           reason="PE warm-up precedes conv")
    nc.compile()
    return nc


def _get_nc():
    if "nc" not in _nc_cache:
        _nc_cache["nc"] = _build_nc()
    return _nc_cache["nc"]


def kernel(x, w, gamma, beta, running_mean, running_var, _trace=False):
    global LAST_RESULTS
    from concourse.bass_utils import run_bass_kernel_spmd

    x = np.ascontiguousarray(np.asarray(x, dtype=np.float32))
    w = np.asarray(w, dtype=np.float32)
    gamma = np.asarray(gamma, dtype=np.float32)
    beta = np.asarray(beta, dtype=np.float32)
    running_mean = np.asarray(running_mean, dtype=np.float32)
    running_var = np.asarray(running_var, dtype=np.float32)

    # fold BN (inference) into per-channel scale/shift
    eps = 1e-5
    scale = gamma / np.sqrt(running_var + eps)
    shift = beta - running_mean * scale

    # weights -> lhsT layout [ci_t, co_t, ci, (ky,kx), co], bf16
    # (optionally hi/lo split)
    wt_f32 = (w.transpose(1, 2, 3, 0)            # [ci, ky, kx, co]
              .reshape(2, 128, 9, 2, 128)        # [ci_t, ci, tap, co_t, co]
              .transpose(0, 3, 1, 2, 4))         # [ci_t, co_t, ci, tap, co]
    parts = []
    rem = wt_f32
    for _ in range(N_SPLIT):
        p = rem.astype(ml_dtypes.bfloat16)
        parts.append(p)
        rem = rem - p.astype(np.float32)
    wt = np.ascontiguousarray(np.stack(parts, axis=0))

    nc = _get_nc()
    bnp = np.ascontiguousarray(
        np.stack([scale, shift], axis=-1).reshape(2, 128, 2).astype(np.float32))
    in_maps = [
        {
            "x": np.ascontiguousarray(x[i * NB:(i + 1) * NB]),
            "wt": wt,
            "bnp": bnp,
        }
        for i in range(N_CORES)
    ]
    res = run_bass_kernel_spmd(nc, in_maps, core_ids=list(range(N_CORES)),
                               trace=_trace)
    LAST_RESULTS = res
    y = np.concatenate([r["y"] for r in res.results], axis=0)
    return y
